# revision 51
# baseline (speedup 1.0000x reference)
"""Causal self-attention (B=2, S=2048, E=1024, H=16, DH=64) on 8 trn2 cores.

Sharding: core c -> (batch b = c//4, head-group g = c%4, heads 4g..4g+3).
Data parallel over batch, tensor parallel over heads, row-sharded Wo;
partial outputs summed on host.

Per-core device kernel (bf16 matmuls, fp32 accumulation), fully software-
pipelined so the PE never starves while the ACT engine grinds through exp:

  phase A (projection units, emitted as PE filler inside the attention loop):
    kqT = (x @ Wkq)^T via W-stationary matmuls on xT (+bias on DVE/Pool),
    v   =  x @ Wv (natural [sk, dh] layout + ones column for row-sums)
  phase B (per chunk c of 512 sq, per head-pair p):
    scores^T[sk,sq] = k q^T (2 heads row-packed in PE via tile_position, K=64)
    P^T = exp(scores/8) on ACT only (causal-trimmed, triu-masked diag)
    AV: sa[sq,b][dh+1] += P^T-block^T @ v_aug  (P^T as stationary operand:
        M=128 sq rows fully used, N=65 -> MAC-optimal, half the cost of the
        v-stationary form; col 64 accumulates the softmax row-sums)
    eviction: per-partition reciprocal of row-sums, normalize-on-evict
    (DVE tensor_scalar / ACT Copy-with-scale), then PE-transpose the
    [sq, 2*64] block back to feature-major saT for the output projection.
  phase C: out[t] = saT^T @ Wo (row-shard), f32, DMA to DRAM; tiles emitted
    early (as soon as their saT rows exist) so the DMA never tails.
"""
import numpy as np
import ml_dtypes

import concourse.bass as bass
import concourse.bacc as bacc
import concourse.tile as tile
from concourse import mybir
from concourse.masks import make_upper_triangular, make_identity

BF16 = mybir.dt.bfloat16
F32 = mybir.dt.float32
NP_BF16 = ml_dtypes.bfloat16

B, S, E, H, DH = 2, 2048, 1024, 16, 64
N_CORES = 8
HPC = 4          # heads per core
SCH = 4          # number of 512-wide sq chunks
SKT = 16         # number of 128-wide sk tiles
ET = 8           # number of 128-wide e tiles

Exp = mybir.ActivationFunctionType.Exp
Copy = mybir.ActivationFunctionType.Copy


def build_nc(reps=1):
    nc = bacc.Bacc(None, target_bir_lowering=False)

    xT = nc.dram_tensor("xT", [E, S], BF16, kind="ExternalInput")
    wkq = nc.dram_tensor("wkq", [E, 512], BF16, kind="ExternalInput")
    wv = nc.dram_tensor("wv", [E, 256], BF16, kind="ExternalInput")
    wo = nc.dram_tensor("wo", [256, E], BF16, kind="ExternalInput")
    bkq = nc.dram_tensor("bkq", [128, 4], F32, kind="ExternalInput")
    out = nc.dram_tensor("out", [S, E], BF16, kind="ExternalOutput")

    with tile.TileContext(nc) as tc:
        import contextlib
        with contextlib.ExitStack() as ctx:
            const = ctx.enter_context(tc.tile_pool(name="const", bufs=1))
            wpool = ctx.enter_context(tc.tile_pool(name="wpool", bufs=1))
            xpool = ctx.enter_context(tc.tile_pool(name="xpool", bufs=1))
            kqpool = ctx.enter_context(tc.tile_pool(name="kqpool", bufs=1))
            vpool = ctx.enter_context(tc.tile_pool(name="vpool", bufs=1))
            sapool = ctx.enter_context(tc.tile_pool(name="sapool", bufs=1))
            pt_pool = ctx.enter_context(tc.tile_pool(name="pt", bufs=34))
            sasb_pool = ctx.enter_context(tc.tile_pool(name="sasb", bufs=2))
            small = ctx.enter_context(tc.tile_pool(name="small", bufs=8))
            ostage = ctx.enter_context(tc.tile_pool(name="ostage", bufs=3))

            # load weights + x first; DMAs split across the SP and ACT hwdge
            # queues so both stream in parallel. Few, large DMAs (issue costs
            # ~0.7-1.2us of sequencer time each); e-tiles 0-3 of wkq/xT land
            # first so the e-major prologue can start after ~2us.
            wkq_sb = wpool.tile([128, ET, 512], BF16)
            xT_sb = xpool.tile([128, ET, S], BF16)
            wv_sb = wpool.tile([128, ET, 256], BF16)
            wkq_r = wkq.rearrange("(n p) f -> p n f", p=128)
            xT_r = xT.rearrange("(n p) s -> p n s", p=128)
            nc.sync.dma_start(wkq_sb[:, 0:4, :], wkq_r[:, 0:4, :])
            nc.scalar.dma_start(wv_sb[:], wv.rearrange("(n p) f -> p n f", p=128))
            nc.scalar.dma_start(xT_sb[:, 0:4, 0:512], xT_r[:, 0:4, 0:512])
            nc.sync.dma_start(xT_sb[:, 4:8, 0:512], xT_r[:, 4:8, 0:512])
            nc.scalar.dma_start(wkq_sb[:, 4:8, :], wkq_r[:, 4:8, :])
            bkq_sb = const.tile([128, 4], F32)
            nc.sync.dma_start(bkq_sb[:], bkq[:])
            for c in range(1, SCH):
                q = nc.sync if c % 2 == 1 else nc.scalar
                q.dma_start(xT_sb[:, :, 512 * c:512 * (c + 1)],
                            xT_r[:, :, 512 * c:512 * (c + 1)])
            wo_sb = wpool.tile([128, 2, E], BF16)
            nc.scalar.dma_start(wo_sb[:], wo.rearrange("(n p) f -> p n f", p=128))

            triu2 = const.tile([128, 2, 128], BF16)
            make_upper_triangular(nc, triu2[:, 0, :], val=1.0, diag=True)
            make_upper_triangular(nc, triu2[:, 1, :], val=1.0, diag=True)
            ident = const.tile([128, 128], BF16)
            make_identity(nc, ident[:])

            kqT_sb = kqpool.tile([128, 4, S], BF16)      # blk: p0k,p0q,p1k,p1q
            vaug_sb = vpool.tile([128, SKT, HPC, 65], BF16)
            nc.vector.memset(vaug_sb[:, :, :, 64:65], 1.0)
            saT_sb = sapool.tile([128, 2, S], BF16)      # dim1: pair

            for _rep in range(reps):
              pools = {}
              def get_pool(name, bufs):
                  if name not in pools:
                      pools[name] = tc.alloc_tile_pool(name=name, bufs=bufs,
                                                       space="PSUM")
                  return pools[name]
              if True:
                # prologue pools (released before the attention loop)
                pre_kq = get_pool("pre_kq", 4)    # [128,512] f32 -> 1 bank x4
                pre_v = get_pool("pre_v", 4)      # [128,256] f32 -> 1 bank x4

                # ---- projection / output units (PE filler) ----
                ot_cur = [None]

                def kq_unit(blk, c, pool, eng=None):
                    ps = pool.tile([128, 512], F32, tag="pj", name="kqps")
                    f0 = 512 * c
                    for e in range(ET):
                        nc.tensor.matmul(
                            ps[:], wkq_sb[:, e, 128 * blk:128 * (blk + 1)],
                            xT_sb[:, e, f0:f0 + 512],
                            start=(e == 0), stop=(e == ET - 1))
                    if eng is nc.scalar:
                        nc.scalar.activation(
                            kqT_sb[:, blk, f0:f0 + 512], ps[:],
                            mybir.ActivationFunctionType.Identity,
                            bias=bkq_sb[:, blk:blk + 1])
                    else:
                        nc.vector.tensor_scalar_add(
                            kqT_sb[:, blk, f0:f0 + 512], ps[:],
                            bkq_sb[:, blk:blk + 1])

                def v_unit(t, pool, eng=None):
                    ps = pool.tile([128, 256], F32, tag="pj", name="vps")
                    for e in range(ET):
                        nc.tensor.matmul(
                            ps[:], xT_sb[:, e, 128 * t:128 * (t + 1)],
                            wv_sb[:, e, :],
                            start=(e == 0), stop=(e == ET - 1))
                    if eng is nc.scalar:
                        nc.scalar.copy(
                            vaug_sb[:, t, :, 0:64],
                            ps[:].rearrange("p (h d) -> p h d", h=HPC))
                    else:
                        nc.vector.tensor_copy(
                            vaug_sb[:, t, :, 0:64],
                            ps[:].rearrange("p (h d) -> p h d", h=HPC))

                def out_unit(t, h, pool, tail=False):
                    # half-tile of the output projection: [128, 512]
                    ps = pool.tile([128, 512], F32, tag="pj", name="ops")
                    f0 = 512 * h
                    nc.tensor.matmul(ps[:],
                                     saT_sb[:, 0, 128 * t:128 * (t + 1)],
                                     wo_sb[:, 0, f0:f0 + 512],
                                     start=True, stop=False)
                    nc.tensor.matmul(ps[:],
                                     saT_sb[:, 1, 128 * t:128 * (t + 1)],
                                     wo_sb[:, 1, f0:f0 + 512],
                                     start=False, stop=True)
                    if h == 0:
                        ot = ostage.tile([128, 2, 512], BF16, tag="ot",
                                         name="ot")
                        ot_cur[0] = ot
                    ot = ot_cur[0]
                    if tail and h == 1:
                        nc.scalar.copy(ot[:, h, :], ps[:])
                    else:
                        nc.vector.tensor_copy(ot[:, h, :], ps[:])
                    if h == 1:
                        q = nc.scalar if tail else nc.sync
                        q.dma_start(
                            out[128 * t:128 * (t + 1), :],
                            ot.rearrange("p n f -> p (n f)"))

                # filler queue (PE work units) + deferred-eviction queue
                filler = []
                evq = []

                def drain(k):
                    for _ in range(k):
                        if filler:
                            filler.pop(0)()

                def pop_ev(k=1):
                    for _ in range(k):
                        if evq:
                            evq.pop(0)()

                # ---- attention segment ----
                # The j-loop only computes scores+exp (ACT-bound); the AV
                # contraction runs later as 4 "waves" (one per 128-row sq
                # block b), emitted as PE filler inside the NEXT segment's
                # j-loop. Each wave owns two psum BANKS (one accumulation
                # group per head) since psum groups are bank-granular.
                def attn_segment(c, p):
                    sq0 = 512 * c
                    kblk, qblk = 2 * p, 2 * p + 1
                    nj = 4 * c + 4
                    pts = []
                    for j in range(nj):
                        r = j - 4 * c
                        diag = r >= 0
                        off = 128 * r if diag else 0
                        w = 512 - off
                        st = stsa_ps.tile([128, 2, 512], F32, tag="st",
                                          name="st")
                        nc.tensor.matmul(
                            st[:, 0, 0:w],
                            kqT_sb[0:64, kblk, 128 * j:128 * (j + 1)],
                            kqT_sb[0:64, qblk, sq0 + off:sq0 + 512],
                            start=True, stop=True, tile_position=(0, 0))
                        nc.tensor.matmul(
                            st[:, 1, 0:w],
                            kqT_sb[64:128, kblk, 128 * j:128 * (j + 1)],
                            kqT_sb[64:128, qblk, sq0 + off:sq0 + 512],
                            start=True, stop=True, tile_position=(64, 0))
                        pt = pt_pool.tile([128, 2, 512], BF16, tag="pt",
                                          name="pt")
                        nc.scalar.activation(pt[:, :, off:512],
                                             st[:, :, 0:w],
                                             Exp, scale=0.125)
                        if diag:
                            nc.gpsimd.tensor_mul(
                                pt[:, :, off:off + 128],
                                pt[:, :, off:off + 128], triu2[:])
                        pts.append(pt)
                        pop_ev()
                        drain(2 if len(filler) > nj - 1 - j else 1)

                    def wave(b):
                        # sa: [128, s, bank] with the per-head accumulation
                        # group in cols 0:65 of its own bank (col 64 = the
                        # softmax row-sums via vaug's ones column)
                        sa = wave_pool[0].tile([128, 2, 512], F32, tag="st",
                                               name="sa")
                        njb = 4 * c + b + 1
                        for s in (0, 1):
                            for j in range(njb):
                                nc.tensor.matmul(
                                    sa[:, s, 0:65],
                                    pts[j][:, s, 128 * b:128 * (b + 1)],
                                    vaug_sb[:, j, 2 * p + s, :],
                                    start=(j == 0), stop=(j == njb - 1))
                        rc = small.tile([128, 2, 1], F32, tag="rc",
                                        name="rc")
                        nc.vector.reciprocal(rc[:], sa[:, :, 64:65])
                        sb = sasb_pool.tile([128, 2, 64], BF16,
                                            tag="sasb", name="sasb")
                        for s in (0, 1):
                            nc.vector.tensor_scalar_mul(
                                sb[:, s, :], sa[:, s, 0:64], rc[:, s, :])
                        tr = tr_ps.tile([128, 128], BF16, tag="tr",
                                        name="tr")
                        nc.tensor.transpose(tr[:], sb[:, :, :], ident[:])
                        nc.vector.tensor_copy(
                            saT_sb[:, p, sq0 + 128 * b:sq0 + 128 * (b + 1)],
                            tr[:])
                    for b in range(4):
                        evq.append(lambda bb=b: wave(bb))

                # ---- schedule ----
                # prologue: chunk-0 projections, e-major so each arriving
                # e-tile feeds 8 matmuls; kq blk0/1 finish + evict first
                # (blk1 via the idle ACT) so the first scores start ASAP
                kps = [pre_kq.tile([128, 512], F32, tag="pj",
                                   name=f"kq{blk}") for blk in range(4)]
                vps = [pre_v.tile([128, 256], F32, tag="pj",
                                  name=f"v{t}") for t in range(4)]
                for e in range(ET):
                    order = range(4) if e < ET - 1 else (0, 1)
                    for blk in order:
                        nc.tensor.matmul(
                            kps[blk][:],
                            wkq_sb[:, e, 128 * blk:128 * (blk + 1)],
                            xT_sb[:, e, 0:512],
                            start=(e == 0), stop=(e == ET - 1))
                    if e < ET - 1:
                        for t in range(4):
                            nc.tensor.matmul(
                                vps[t][:],
                                xT_sb[:, e, 128 * t:128 * (t + 1)],
                                wv_sb[:, e, :],
                                start=(e == 0), stop=False)
                nc.vector.tensor_scalar_add(
                    kqT_sb[:, 0, 0:512], kps[0][:], bkq_sb[:, 0:1])
                nc.scalar.activation(
                    kqT_sb[:, 1, 0:512], kps[1][:],
                    mybir.ActivationFunctionType.Identity,
                    bias=bkq_sb[:, 1:2])
                for blk in (2, 3):
                    nc.tensor.matmul(
                        kps[blk][:],
                        wkq_sb[:, ET - 1, 128 * blk:128 * (blk + 1)],
                        xT_sb[:, ET - 1, 0:512],
                        start=False, stop=True)
                for t in range(4):
                    nc.tensor.matmul(
                        vps[t][:], xT_sb[:, ET - 1, 128 * t:128 * (t + 1)],
                        wv_sb[:, ET - 1, :], start=False, stop=True)
                nc.vector.tensor_scalar_add(
                    kqT_sb[:, 2, 0:512], kps[2][:], bkq_sb[:, 2:3])
                nc.scalar.activation(
                    kqT_sb[:, 3, 0:512], kps[3][:],
                    mybir.ActivationFunctionType.Identity,
                    bias=bkq_sb[:, 3:4])
                for t in range(4):
                    eng_v = nc.vector if t % 2 == 0 else nc.scalar
                    if eng_v is nc.scalar:
                        nc.scalar.copy(
                            vaug_sb[:, t, :, 0:64],
                            vps[t][:].rearrange("p (h d) -> p h d", h=HPC))
                    else:
                        nc.vector.tensor_copy(
                            vaug_sb[:, t, :, 0:64],
                            vps[t][:].rearrange("p (h d) -> p h d", h=HPC))
                pools.pop("pre_v").release()
                pools.pop("pre_kq").release()

                pj_ps = get_pool("pj_ps", 1)     # 1 x 1 bank
                tr_ps = get_pool("tr_ps", 1)     # 1 x 1 bank
                stsa_ps = get_pool("stsa_ps", 3)  # 3 x 2 banks (st + AV waves)
                wave_pool = [stsa_ps]

                for c in range(SCH):
                    if c + 1 < SCH:
                        for blk in range(4):
                            filler.append(
                                lambda b=blk, cc=c + 1:
                                kq_unit(b, cc, pj_ps))
                        for t in range(4 * c + 4, 4 * c + 8):
                            filler.append(
                                lambda tt=t: v_unit(tt, pj_ps))
                        if c == 2:
                            for t in range(0, 4):
                                for h in range(2):
                                    filler.append(
                                        lambda tt=t, hh=h:
                                        out_unit(tt, hh, pj_ps))
                    else:
                        for t in range(4, 8):
                            for h in range(2):
                                filler.append(
                                    lambda tt=t, hh=h:
                                    out_unit(tt, hh, pj_ps))
                    attn_segment(c, 0)
                    if c == 3:
                        for t in range(8, 12):
                            for h in range(2):
                                filler.append(
                                    lambda tt=t, hh=h:
                                    out_unit(tt, hh, pj_ps))
                    attn_segment(c, 1)
                    drain(len(filler))  # anything not yet emitted
                # tail: release the attention pools, redo the last 4 AV waves
                # and output tiles with fat double-buffered pools, waves
                # interleaved with out tiles (tile 12+b only needs wave b)
                pools.pop("stsa_ps").release()
                tail_sa = get_pool("tail_sa", 2)   # [128,2,512] f32 -> 2x2
                tail_pj = get_pool("tail_pj", 2)   # 2 x 1 bank
                wave_pool[0] = tail_sa
                pop_ev()
                for b in range(4):
                    pop_ev()
                    for h in range(2):
                        out_unit(12 + b, h, tail_pj, tail=True)

              for pl in reversed(list(pools)):
                  pools.pop(pl).release()

    nc.compile()
    return nc


_CACHE = {}


def _build_runner():
    """Build the SPMD PJRT executable once; returns a dict with a jitted fn."""
    import jax
    from jax.sharding import Mesh, PartitionSpec
    from jax.experimental.shard_map import shard_map
    from concourse import bass2jax as b2j
    from concourse import mybir as _mybir

    if "runner" in _CACHE:
        return _CACHE["runner"]

    nc = _CACHE.get("nc")
    if nc is None:
        nc = _CACHE["nc"] = build_nc()

    b2j.install_neuronx_cc_hook()
    partition_name = (nc.partition_id_tensor.name
                      if nc.partition_id_tensor else None)

    in_names, out_names, out_avals = [], [], []
    for alloc in nc.m.functions[0].allocations:
        if not isinstance(alloc, _mybir.MemoryLocationSet):
            continue
        name = alloc.memorylocations[0].name
        if alloc.kind == "ExternalInput":
            if name != partition_name:
                in_names.append(name)
        elif alloc.kind == "ExternalOutput":
            out_names.append(name)
            out_avals.append(jax.core.ShapedArray(
                tuple(alloc.tensor_shape), _mybir.dt.np(alloc.dtype)))
    n_params = len(in_names)
    zero_out_shapes = [(a.shape, a.dtype) for a in out_avals]
    all_in_names = list(in_names) + list(out_names)
    if partition_name is not None:
        all_in_names.append(partition_name)

    def _body(*args):
        operands = list(args)
        if partition_name is not None:
            operands.append(b2j.partition_id_tensor())
        outs = b2j._bass_exec_p.bind(
            *operands,
            out_avals=tuple(out_avals),
            in_names=tuple(all_in_names),
            out_names=tuple(out_names),
            lowering_input_output_aliases=(),
            sim_require_finite=True,
            sim_require_nnan=True,
            nc=nc,
        )
        return tuple(outs)

    devices = jax.devices()[:N_CORES]
    mesh = Mesh(np.asarray(devices), ("core",))
    n_outs = len(out_names)
    in_specs = (PartitionSpec("core"),) * (n_params + n_outs)
    out_specs = (PartitionSpec("core"),) * n_outs
    fn = jax.jit(shard_map(_body, mesh=mesh, in_specs=in_specs,
                           out_specs=out_specs, check_rep=False),
                 keep_unused=True)
    runner = {
        "fn": fn,
        "in_names": in_names,
        "out_names": out_names,
        "out_avals": out_avals,
        "zero_out_shapes": zero_out_shapes,
        "mesh": mesh,
    }
    _CACHE["runner"] = runner
    return runner


def _run_spmd(in_maps):
    """Execute on 8 cores, returning list of per-core output dicts."""
    r = _build_runner()
    n_cores = N_CORES
    concat_in = [
        np.concatenate([np.asarray(in_maps[c][name]) for c in range(n_cores)],
                       axis=0)
        for name in r["in_names"]
    ]
    if "zeros" not in r:
        r["zeros"] = [np.zeros((n_cores * s[0], *s[1:]), d)
                      for s, d in r["zero_out_shapes"]]
    out_arrs = r["fn"](*concat_in, *r["zeros"])
    return [
        {name: np.asarray(out_arrs[i]).reshape(n_cores, *r["out_avals"][i].shape)[c]
         for i, name in enumerate(r["out_names"])}
        for c in range(n_cores)
    ]


def _prep_core_inputs(x, Wkqv, bkqv, Wo):
    """Host-side shard/pack. Returns (in_maps, host_bias) for 8 cores."""
    xT = [np.ascontiguousarray(x[b].T).astype(NP_BF16) for b in range(B)]
    per_g = []
    for g in range(4):
        h0 = 4 * g
        wkq = np.empty((E, 512), np.float32)
        for p in range(2):
            a, b_ = h0 + 2 * p, h0 + 2 * p + 1
            wkq[:, 256 * p:256 * p + 64] = Wkqv[a][:, 0:64]
            wkq[:, 256 * p + 64:256 * p + 128] = Wkqv[b_][:, 0:64]
            wkq[:, 256 * p + 128:256 * p + 192] = Wkqv[a][:, 64:128]
            wkq[:, 256 * p + 192:256 * p + 256] = Wkqv[b_][:, 64:128]
        wv = np.concatenate([Wkqv[h0 + h][:, 128:192] for h in range(HPC)],
                            axis=1)
        wog = Wo[256 * g:256 * (g + 1), :]
        bkq_arr = np.empty((128, 4), np.float32)
        for p in range(2):
            a, b_ = h0 + 2 * p, h0 + 2 * p + 1
            bkq_arr[0:64, 2 * p] = bkqv[a][0:64]
            bkq_arr[64:128, 2 * p] = bkqv[b_][0:64]
            bkq_arr[0:64, 2 * p + 1] = bkqv[a][64:128]
            bkq_arr[64:128, 2 * p + 1] = bkqv[b_][64:128]
        per_g.append({
            "wkq": wkq.astype(NP_BF16),
            "wv": wv.astype(NP_BF16),
            "wo": wog.astype(NP_BF16),
            "bkq": bkq_arr,
        })
    in_maps = []
    for c in range(N_CORES):
        b, g = c // 4, c % 4
        m = dict(per_g[g])
        m["xT"] = xT[b]
        in_maps.append(m)
    bv = np.concatenate([bkqv[h][128:192] for h in range(H)])
    return in_maps, bv


def kernel(x, Wkqv, bkqv, Wo, bo):
    x = np.asarray(x, np.float32)
    Wkqv = np.asarray(Wkqv, np.float32)
    bkqv = np.asarray(bkqv, np.float32)
    Wo = np.asarray(Wo, np.float32)
    bo = np.asarray(bo, np.float32)

    in_maps, bv = _prep_core_inputs(x, Wkqv, bkqv, Wo)
    results = _run_spmd(in_maps)
    partials = np.stack([results[c]["out"] for c in range(N_CORES)]).astype(np.float32)
    partials = partials.reshape(B, 4, S, E).sum(axis=1)
    base = bv @ Wo + bo
    return (partials + base[None, None, :]).astype(np.float32)


# revision 52
# speedup vs baseline: 1.0052x; 1.0052x over previous
"""Causal self-attention (B=2, S=2048, E=1024, H=16, DH=64) on 8 trn2 cores.

Sharding: core c -> (batch b = c//4, head-group g = c%4, heads 4g..4g+3).
Data parallel over batch, tensor parallel over heads, row-sharded Wo;
partial outputs summed on host.

Per-core device kernel (bf16 matmuls, fp32 accumulation), fully software-
pipelined so the PE never starves while the ACT engine grinds through exp:

  - scores^T[sk,sq] = k q^T per (512-wide sq chunk c, head-pair p, sk tile j),
    2 heads row-packed in the PE via tile_position (K=64); the j-loop emits
    ONLY scores+exp (ACT-bound) and keeps all pt = exp(scores/8) tiles of a
    segment resident in SBUF (causal-trimmed, triu diag masks on Pool).
  - The AV contraction runs as 4 deferred "waves" per segment (one per
    128-row sq block b), emitted as PE filler inside the NEXT segment's
    j-loop: sa[128sq, 65] += pt-block^T @ v_aug with pt stationary (M=128
    fully used, N=65 -> MAC-optimal, half the cost of the v-stationary
    form; col 64 accumulates softmax row-sums via v_aug's ones column).
    Each wave owns two whole psum BANKS (psum accumulation groups are
    2KB-bank-granular). Eviction: per-partition reciprocal of the row-sums,
    normalize-on-evict (DVE tensor_scalar), one PE-transpose per block
    (2 heads packed) back to feature-major saT.
  - Projections (kqT = (x Wkq)^T + b, v = x Wv) and output tiles
    (out[t] = saT^T @ Wo row-shard, bf16) are emitted as paced PE filler
    units between attention j-iterations; an e-major prologue overlaps the
    first chunk's projections with the input DMAs (split across the SP and
    ACT hwdge queues); the tail interleaves the last AV waves with the
    final output tiles on fat double-buffered pools.
"""
import numpy as np
import ml_dtypes

import concourse.bass as bass
import concourse.bacc as bacc
import concourse.tile as tile
from concourse import mybir
from concourse.masks import make_upper_triangular, make_identity

BF16 = mybir.dt.bfloat16
F32 = mybir.dt.float32
NP_BF16 = ml_dtypes.bfloat16

B, S, E, H, DH = 2, 2048, 1024, 16, 64
N_CORES = 8
HPC = 4          # heads per core
SCH = 4          # number of 512-wide sq chunks
SKT = 16         # number of 128-wide sk tiles
ET = 8           # number of 128-wide e tiles

Exp = mybir.ActivationFunctionType.Exp
Copy = mybir.ActivationFunctionType.Copy


def build_nc(reps=1):
    nc = bacc.Bacc(None, target_bir_lowering=False)

    xT = nc.dram_tensor("xT", [E, S], BF16, kind="ExternalInput")
    wkq = nc.dram_tensor("wkq", [E, 512], BF16, kind="ExternalInput")
    wv = nc.dram_tensor("wv", [E, 256], BF16, kind="ExternalInput")
    wo = nc.dram_tensor("wo", [256, E], BF16, kind="ExternalInput")
    bkq = nc.dram_tensor("bkq", [128, 4], F32, kind="ExternalInput")
    out = nc.dram_tensor("out", [S, E], BF16, kind="ExternalOutput")

    with tile.TileContext(nc) as tc:
        import contextlib
        with contextlib.ExitStack() as ctx:
            const = ctx.enter_context(tc.tile_pool(name="const", bufs=1))
            wpool = ctx.enter_context(tc.tile_pool(name="wpool", bufs=1))
            xpool = ctx.enter_context(tc.tile_pool(name="xpool", bufs=1))
            kqpool = ctx.enter_context(tc.tile_pool(name="kqpool", bufs=1))
            vpool = ctx.enter_context(tc.tile_pool(name="vpool", bufs=1))
            sapool = ctx.enter_context(tc.tile_pool(name="sapool", bufs=1))
            pt_pool = ctx.enter_context(tc.tile_pool(name="pt", bufs=34))
            sasb_pool = ctx.enter_context(tc.tile_pool(name="sasb", bufs=2))
            small = ctx.enter_context(tc.tile_pool(name="small", bufs=8))
            ostage = ctx.enter_context(tc.tile_pool(name="ostage", bufs=3))

            # load weights + x first; DMAs split across the SP and ACT hwdge
            # queues so both stream in parallel. Few, large DMAs (issue costs
            # ~0.7-1.2us of sequencer time each); e-tiles 0-3 of wkq/xT land
            # first so the e-major prologue can start after ~2us.
            wkq_sb = wpool.tile([128, ET, 512], BF16)
            xT_sb = xpool.tile([128, ET, S], BF16)
            wv_sb = wpool.tile([128, ET, 256], BF16)
            wkq_r = wkq.rearrange("(n p) f -> p n f", p=128)
            xT_r = xT.rearrange("(n p) s -> p n s", p=128)
            nc.sync.dma_start(wkq_sb[:, 0:4, :], wkq_r[:, 0:4, :])
            nc.scalar.dma_start(wv_sb[:], wv.rearrange("(n p) f -> p n f", p=128))
            nc.scalar.dma_start(xT_sb[:, 0:4, 0:512], xT_r[:, 0:4, 0:512])
            nc.sync.dma_start(xT_sb[:, 4:8, 0:512], xT_r[:, 4:8, 0:512])
            nc.scalar.dma_start(wkq_sb[:, 4:8, :], wkq_r[:, 4:8, :])
            bkq_sb = const.tile([128, 4], F32)
            nc.sync.dma_start(bkq_sb[:], bkq[:])
            for c in range(1, SCH):
                q = nc.sync if c % 2 == 1 else nc.scalar
                q.dma_start(xT_sb[:, :, 512 * c:512 * (c + 1)],
                            xT_r[:, :, 512 * c:512 * (c + 1)])
            wo_sb = wpool.tile([128, 2, E], BF16)
            nc.scalar.dma_start(wo_sb[:], wo.rearrange("(n p) f -> p n f", p=128))

            triu2 = const.tile([128, 2, 128], BF16)
            make_upper_triangular(nc, triu2[:, 0, :], val=1.0, diag=True)
            make_upper_triangular(nc, triu2[:, 1, :], val=1.0, diag=True)
            ident = const.tile([128, 128], BF16)
            make_identity(nc, ident[:])

            kqT_sb = kqpool.tile([128, 4, S], BF16)      # blk: p0k,p0q,p1k,p1q
            vaug_sb = vpool.tile([128, SKT, HPC, 65], BF16)
            nc.vector.memset(vaug_sb[:, :, :, 64:65], 1.0)
            saT_sb = sapool.tile([128, 2, S], BF16)      # dim1: pair

            for _rep in range(reps):
              pools = {}
              def get_pool(name, bufs):
                  if name not in pools:
                      pools[name] = tc.alloc_tile_pool(name=name, bufs=bufs,
                                                       space="PSUM")
                  return pools[name]
              if True:
                # prologue pools (released before the attention loop)
                pre_kq = get_pool("pre_kq", 4)    # [128,512] f32 -> 1 bank x4
                pre_v = get_pool("pre_v", 4)      # [128,256] f32 -> 1 bank x4

                # ---- projection / output units (PE filler) ----
                ot_cur = [None]

                def kq_unit(blk, c, pool, eng=None):
                    ps = pool.tile([128, 512], F32, tag="pj", name="kqps")
                    f0 = 512 * c
                    for e in range(ET):
                        nc.tensor.matmul(
                            ps[:], wkq_sb[:, e, 128 * blk:128 * (blk + 1)],
                            xT_sb[:, e, f0:f0 + 512],
                            start=(e == 0), stop=(e == ET - 1))
                    if eng is nc.scalar:
                        nc.scalar.activation(
                            kqT_sb[:, blk, f0:f0 + 512], ps[:],
                            mybir.ActivationFunctionType.Identity,
                            bias=bkq_sb[:, blk:blk + 1])
                    else:
                        nc.vector.tensor_scalar_add(
                            kqT_sb[:, blk, f0:f0 + 512], ps[:],
                            bkq_sb[:, blk:blk + 1])

                def v_unit(t, pool, eng=None):
                    ps = pool.tile([128, 256], F32, tag="pj", name="vps")
                    for e in range(ET):
                        nc.tensor.matmul(
                            ps[:], xT_sb[:, e, 128 * t:128 * (t + 1)],
                            wv_sb[:, e, :],
                            start=(e == 0), stop=(e == ET - 1))
                    if eng is nc.scalar:
                        nc.scalar.copy(
                            vaug_sb[:, t, :, 0:64],
                            ps[:].rearrange("p (h d) -> p h d", h=HPC))
                    else:
                        nc.vector.tensor_copy(
                            vaug_sb[:, t, :, 0:64],
                            ps[:].rearrange("p (h d) -> p h d", h=HPC))

                def out_unit(t, h, pool, tail=False):
                    # half-tile of the output projection: [128, 512]
                    ps = pool.tile([128, 512], F32, tag="pj", name="ops")
                    f0 = 512 * h
                    nc.tensor.matmul(ps[:],
                                     saT_sb[:, 0, 128 * t:128 * (t + 1)],
                                     wo_sb[:, 0, f0:f0 + 512],
                                     start=True, stop=False)
                    nc.tensor.matmul(ps[:],
                                     saT_sb[:, 1, 128 * t:128 * (t + 1)],
                                     wo_sb[:, 1, f0:f0 + 512],
                                     start=False, stop=True)
                    if h == 0:
                        ot = ostage.tile([128, 2, 512], BF16, tag="ot",
                                         name="ot")
                        ot_cur[0] = ot
                    ot = ot_cur[0]
                    if tail and h == 1:
                        nc.scalar.copy(ot[:, h, :], ps[:])
                    else:
                        nc.vector.tensor_copy(ot[:, h, :], ps[:])
                    if h == 1:
                        q = nc.scalar if tail else nc.sync
                        q.dma_start(
                            out[128 * t:128 * (t + 1), :],
                            ot.rearrange("p n f -> p (n f)"))

                # filler queue (PE work units) + deferred-eviction queue
                filler = []
                evq = []

                def drain(k):
                    for _ in range(k):
                        if filler:
                            filler.pop(0)()

                def pop_ev(k=1):
                    for _ in range(k):
                        if evq:
                            evq.pop(0)()

                # ---- attention segment ----
                # The j-loop only computes scores+exp (ACT-bound); the AV
                # contraction runs later as 4 "waves" (one per 128-row sq
                # block b), emitted as PE filler inside the NEXT segment's
                # j-loop. Each wave owns two psum BANKS (one accumulation
                # group per head) since psum groups are bank-granular.
                def attn_segment(c, p):
                    sq0 = 512 * c
                    kblk, qblk = 2 * p, 2 * p + 1
                    nj = 4 * c + 4
                    pts = []
                    for j in range(nj):
                        r = j - 4 * c
                        diag = r >= 0
                        off = 128 * r if diag else 0
                        w = 512 - off
                        st = stsa_ps.tile([128, 2, 512], F32, tag="st",
                                          name="st")
                        nc.tensor.matmul(
                            st[:, 0, 0:w],
                            kqT_sb[0:64, kblk, 128 * j:128 * (j + 1)],
                            kqT_sb[0:64, qblk, sq0 + off:sq0 + 512],
                            start=True, stop=True, tile_position=(0, 0))
                        nc.tensor.matmul(
                            st[:, 1, 0:w],
                            kqT_sb[64:128, kblk, 128 * j:128 * (j + 1)],
                            kqT_sb[64:128, qblk, sq0 + off:sq0 + 512],
                            start=True, stop=True, tile_position=(64, 0))
                        pt = pt_pool.tile([128, 2, 512], BF16, tag="pt",
                                          name="pt")
                        nc.scalar.activation(pt[:, :, off:512],
                                             st[:, :, 0:w],
                                             Exp, scale=0.125)
                        if diag:
                            nc.gpsimd.tensor_mul(
                                pt[:, :, off:off + 128],
                                pt[:, :, off:off + 128], triu2[:])
                        pts.append(pt)
                        pop_ev()
                        drain(2 if len(filler) > nj - 1 - j else 1)

                    def wave(b):
                        # sa: [128, s, bank] with the per-head accumulation
                        # group in cols 0:65 of its own bank (col 64 = the
                        # softmax row-sums via vaug's ones column)
                        sa = wave_pool[0].tile([128, 2, 512], F32, tag="st",
                                               name="sa")
                        njb = 4 * c + b + 1
                        for s in (0, 1):
                            for j in range(njb):
                                nc.tensor.matmul(
                                    sa[:, s, 0:65],
                                    pts[j][:, s, 128 * b:128 * (b + 1)],
                                    vaug_sb[:, j, 2 * p + s, :],
                                    start=(j == 0), stop=(j == njb - 1))
                        rc = small.tile([128, 2, 1], F32, tag="rc",
                                        name="rc")
                        nc.vector.reciprocal(rc[:], sa[:, :, 64:65])
                        sb = sasb_pool.tile([128, 2, 64], BF16,
                                            tag="sasb", name="sasb")
                        for s in (0, 1):
                            nc.vector.tensor_scalar_mul(
                                sb[:, s, :], sa[:, s, 0:64], rc[:, s, :])
                        tr = tr_ps.tile([128, 128], BF16, tag="tr",
                                        name="tr")
                        nc.tensor.transpose(tr[:], sb[:, :, :], ident[:])
                        nc.vector.tensor_copy(
                            saT_sb[:, p, sq0 + 128 * b:sq0 + 128 * (b + 1)],
                            tr[:])
                    for b in range(4):
                        evq.append(lambda bb=b: wave(bb))

                # ---- schedule ----
                # prologue: chunk-0 projections, e-major so each arriving
                # e-tile feeds 8 matmuls; kq blk0/1 finish + evict first
                # (blk1 via the idle ACT) so the first scores start ASAP
                kps = [pre_kq.tile([128, 512], F32, tag="pj",
                                   name=f"kq{blk}") for blk in range(4)]
                vps = [pre_v.tile([128, 256], F32, tag="pj",
                                  name=f"v{t}") for t in range(4)]
                for e in range(ET):
                    order = range(4) if e < ET - 1 else (0, 1)
                    for blk in order:
                        nc.tensor.matmul(
                            kps[blk][:],
                            wkq_sb[:, e, 128 * blk:128 * (blk + 1)],
                            xT_sb[:, e, 0:512],
                            start=(e == 0), stop=(e == ET - 1))
                    if e < ET - 1:
                        for t in range(4):
                            nc.tensor.matmul(
                                vps[t][:],
                                xT_sb[:, e, 128 * t:128 * (t + 1)],
                                wv_sb[:, e, :],
                                start=(e == 0), stop=False)
                nc.vector.tensor_scalar_add(
                    kqT_sb[:, 0, 0:512], kps[0][:], bkq_sb[:, 0:1])
                nc.scalar.activation(
                    kqT_sb[:, 1, 0:512], kps[1][:],
                    mybir.ActivationFunctionType.Identity,
                    bias=bkq_sb[:, 1:2])
                for blk in (2, 3):
                    nc.tensor.matmul(
                        kps[blk][:],
                        wkq_sb[:, ET - 1, 128 * blk:128 * (blk + 1)],
                        xT_sb[:, ET - 1, 0:512],
                        start=False, stop=True)
                for t in range(4):
                    nc.tensor.matmul(
                        vps[t][:], xT_sb[:, ET - 1, 128 * t:128 * (t + 1)],
                        wv_sb[:, ET - 1, :], start=False, stop=True)
                nc.vector.tensor_scalar_add(
                    kqT_sb[:, 2, 0:512], kps[2][:], bkq_sb[:, 2:3])
                nc.scalar.activation(
                    kqT_sb[:, 3, 0:512], kps[3][:],
                    mybir.ActivationFunctionType.Identity,
                    bias=bkq_sb[:, 3:4])
                for t in range(4):
                    eng_v = nc.vector if t % 2 == 0 else nc.scalar
                    if eng_v is nc.scalar:
                        nc.scalar.copy(
                            vaug_sb[:, t, :, 0:64],
                            vps[t][:].rearrange("p (h d) -> p h d", h=HPC))
                    else:
                        nc.vector.tensor_copy(
                            vaug_sb[:, t, :, 0:64],
                            vps[t][:].rearrange("p (h d) -> p h d", h=HPC))
                pools.pop("pre_v").release()
                pools.pop("pre_kq").release()

                pj_ps = get_pool("pj_ps", 1)     # 1 x 1 bank
                tr_ps = get_pool("tr_ps", 1)     # 1 x 1 bank
                stsa_ps = get_pool("stsa_ps", 3)  # 3 x 2 banks (st + AV waves)
                wave_pool = [stsa_ps]

                for c in range(SCH):
                    if c + 1 < SCH:
                        for blk in range(4):
                            filler.append(
                                lambda b=blk, cc=c + 1:
                                kq_unit(b, cc, pj_ps))
                        for t in range(4 * c + 4, 4 * c + 8):
                            filler.append(
                                lambda tt=t: v_unit(tt, pj_ps))
                        if c == 2:
                            for t in range(0, 4):
                                for h in range(2):
                                    filler.append(
                                        lambda tt=t, hh=h:
                                        out_unit(tt, hh, pj_ps))
                    else:
                        for t in range(4, 8):
                            for h in range(2):
                                filler.append(
                                    lambda tt=t, hh=h:
                                    out_unit(tt, hh, pj_ps))
                    attn_segment(c, 0)
                    if c == 3:
                        for t in range(8, 12):
                            for h in range(2):
                                filler.append(
                                    lambda tt=t, hh=h:
                                    out_unit(tt, hh, pj_ps))
                    attn_segment(c, 1)
                    drain(len(filler))  # anything not yet emitted
                # tail: release the attention pools, redo the last 4 AV waves
                # and output tiles with fat double-buffered pools, waves
                # interleaved with out tiles (tile 12+b only needs wave b)
                pools.pop("stsa_ps").release()
                tail_sa = get_pool("tail_sa", 2)   # [128,2,512] f32 -> 2x2
                tail_pj = get_pool("tail_pj", 2)   # 2 x 1 bank
                wave_pool[0] = tail_sa
                pop_ev()
                for b in range(4):
                    pop_ev()
                    for h in range(2):
                        out_unit(12 + b, h, tail_pj, tail=True)

              for pl in reversed(list(pools)):
                  pools.pop(pl).release()

    nc.compile()
    return nc


_CACHE = {}


def _build_runner():
    """Build the SPMD PJRT executable once; returns a dict with a jitted fn."""
    import jax
    from jax.sharding import Mesh, PartitionSpec
    from jax.experimental.shard_map import shard_map
    from concourse import bass2jax as b2j
    from concourse import mybir as _mybir

    if "runner" in _CACHE:
        return _CACHE["runner"]

    nc = _CACHE.get("nc")
    if nc is None:
        nc = _CACHE["nc"] = build_nc()

    b2j.install_neuronx_cc_hook()
    partition_name = (nc.partition_id_tensor.name
                      if nc.partition_id_tensor else None)

    in_names, out_names, out_avals = [], [], []
    for alloc in nc.m.functions[0].allocations:
        if not isinstance(alloc, _mybir.MemoryLocationSet):
            continue
        name = alloc.memorylocations[0].name
        if alloc.kind == "ExternalInput":
            if name != partition_name:
                in_names.append(name)
        elif alloc.kind == "ExternalOutput":
            out_names.append(name)
            out_avals.append(jax.core.ShapedArray(
                tuple(alloc.tensor_shape), _mybir.dt.np(alloc.dtype)))
    n_params = len(in_names)
    zero_out_shapes = [(a.shape, a.dtype) for a in out_avals]
    all_in_names = list(in_names) + list(out_names)
    if partition_name is not None:
        all_in_names.append(partition_name)

    def _body(*args):
        operands = list(args)
        if partition_name is not None:
            operands.append(b2j.partition_id_tensor())
        outs = b2j._bass_exec_p.bind(
            *operands,
            out_avals=tuple(out_avals),
            in_names=tuple(all_in_names),
            out_names=tuple(out_names),
            lowering_input_output_aliases=(),
            sim_require_finite=True,
            sim_require_nnan=True,
            nc=nc,
        )
        return tuple(outs)

    devices = jax.devices()[:N_CORES]
    mesh = Mesh(np.asarray(devices), ("core",))
    n_outs = len(out_names)
    in_specs = (PartitionSpec("core"),) * (n_params + n_outs)
    out_specs = (PartitionSpec("core"),) * n_outs
    fn = jax.jit(shard_map(_body, mesh=mesh, in_specs=in_specs,
                           out_specs=out_specs, check_rep=False),
                 keep_unused=True)
    runner = {
        "fn": fn,
        "in_names": in_names,
        "out_names": out_names,
        "out_avals": out_avals,
        "zero_out_shapes": zero_out_shapes,
        "mesh": mesh,
    }
    _CACHE["runner"] = runner
    return runner


def _run_spmd(in_maps):
    """Execute on 8 cores, returning list of per-core output dicts."""
    r = _build_runner()
    n_cores = N_CORES
    concat_in = [
        np.concatenate([np.asarray(in_maps[c][name]) for c in range(n_cores)],
                       axis=0)
        for name in r["in_names"]
    ]
    if "zeros" not in r:
        r["zeros"] = [np.zeros((n_cores * s[0], *s[1:]), d)
                      for s, d in r["zero_out_shapes"]]
    out_arrs = r["fn"](*concat_in, *r["zeros"])
    return [
        {name: np.asarray(out_arrs[i]).reshape(n_cores, *r["out_avals"][i].shape)[c]
         for i, name in enumerate(r["out_names"])}
        for c in range(n_cores)
    ]


def _prep_core_inputs(x, Wkqv, bkqv, Wo):
    """Host-side shard/pack. Returns (in_maps, host_bias) for 8 cores."""
    xT = [np.ascontiguousarray(x[b].T).astype(NP_BF16) for b in range(B)]
    per_g = []
    for g in range(4):
        h0 = 4 * g
        wkq = np.empty((E, 512), np.float32)
        for p in range(2):
            a, b_ = h0 + 2 * p, h0 + 2 * p + 1
            wkq[:, 256 * p:256 * p + 64] = Wkqv[a][:, 0:64]
            wkq[:, 256 * p + 64:256 * p + 128] = Wkqv[b_][:, 0:64]
            wkq[:, 256 * p + 128:256 * p + 192] = Wkqv[a][:, 64:128]
            wkq[:, 256 * p + 192:256 * p + 256] = Wkqv[b_][:, 64:128]
        wv = np.concatenate([Wkqv[h0 + h][:, 128:192] for h in range(HPC)],
                            axis=1)
        wog = Wo[256 * g:256 * (g + 1), :]
        bkq_arr = np.empty((128, 4), np.float32)
        for p in range(2):
            a, b_ = h0 + 2 * p, h0 + 2 * p + 1
            bkq_arr[0:64, 2 * p] = bkqv[a][0:64]
            bkq_arr[64:128, 2 * p] = bkqv[b_][0:64]
            bkq_arr[0:64, 2 * p + 1] = bkqv[a][64:128]
            bkq_arr[64:128, 2 * p + 1] = bkqv[b_][64:128]
        per_g.append({
            "wkq": wkq.astype(NP_BF16),
            "wv": wv.astype(NP_BF16),
            "wo": wog.astype(NP_BF16),
            "bkq": bkq_arr,
        })
    in_maps = []
    for c in range(N_CORES):
        b, g = c // 4, c % 4
        m = dict(per_g[g])
        m["xT"] = xT[b]
        in_maps.append(m)
    bv = np.concatenate([bkqv[h][128:192] for h in range(H)])
    return in_maps, bv


def kernel(x, Wkqv, bkqv, Wo, bo):
    x = np.asarray(x, np.float32)
    Wkqv = np.asarray(Wkqv, np.float32)
    bkqv = np.asarray(bkqv, np.float32)
    Wo = np.asarray(Wo, np.float32)
    bo = np.asarray(bo, np.float32)

    in_maps, bv = _prep_core_inputs(x, Wkqv, bkqv, Wo)
    results = _run_spmd(in_maps)
    partials = np.stack([results[c]["out"] for c in range(N_CORES)]).astype(np.float32)
    partials = partials.reshape(B, 4, S, E).sum(axis=1)
    base = bv @ Wo + bo
    return (partials + base[None, None, :]).astype(np.float32)


# revision 55
# speedup vs baseline: 1.0079x; 1.0027x over previous
"""Causal self-attention (B=2, S=2048, E=1024, H=16, DH=64) on 8 trn2 cores.

Sharding: core c -> (batch b = c//4, head-group g = c%4, heads 4g..4g+3).
Data parallel over batch, tensor parallel over heads, row-sharded Wo;
partial outputs summed on host.

Per-core device kernel (bf16 matmuls, fp32 accumulation), fully software-
pipelined so the PE never starves while the ACT engine grinds through exp:

  - scores^T[sk,sq] = k q^T per (512-wide sq chunk c, head-pair p, sk tile j),
    2 heads row-packed in the PE via tile_position (K=64); the j-loop emits
    ONLY scores+exp (ACT-bound) and keeps all pt = exp(scores/8) tiles of a
    segment resident in SBUF (causal-trimmed, triu diag masks on Pool).
  - The AV contraction runs as 4 deferred "waves" per segment (one per
    128-row sq block b), emitted as PE filler inside the NEXT segment's
    j-loop: sa[128sq, 65] += pt-block^T @ v_aug with pt stationary (M=128
    fully used, N=65 -> MAC-optimal, half the cost of the v-stationary
    form; col 64 accumulates softmax row-sums via v_aug's ones column).
    Each wave owns two whole psum BANKS (psum accumulation groups are
    2KB-bank-granular). Eviction: per-partition reciprocal of the row-sums,
    normalize-on-evict (DVE tensor_scalar), one PE-transpose per block
    (2 heads packed) back to feature-major saT.
  - Projections (kqT = (x Wkq)^T + b, v = x Wv) and output tiles
    (out[t] = saT^T @ Wo row-shard, bf16) are emitted as paced PE filler
    units between attention j-iterations; an e-major prologue overlaps the
    first chunk's projections with the input DMAs (split across the SP and
    ACT hwdge queues); the tail interleaves the last AV waves with the
    final output tiles on fat double-buffered pools.
"""
import numpy as np
import ml_dtypes

import concourse.bass as bass
import concourse.bacc as bacc
import concourse.tile as tile
from concourse import mybir
from concourse.masks import make_upper_triangular, make_identity

BF16 = mybir.dt.bfloat16
F32 = mybir.dt.float32
NP_BF16 = ml_dtypes.bfloat16

B, S, E, H, DH = 2, 2048, 1024, 16, 64
N_CORES = 8
HPC = 4          # heads per core
SCH = 4          # number of 512-wide sq chunks
SKT = 16         # number of 128-wide sk tiles
ET = 8           # number of 128-wide e tiles

Exp = mybir.ActivationFunctionType.Exp
Copy = mybir.ActivationFunctionType.Copy


def build_nc(reps=1):
    nc = bacc.Bacc(None, target_bir_lowering=False)

    xT = nc.dram_tensor("xT", [E, S], BF16, kind="ExternalInput")
    wkq = nc.dram_tensor("wkq", [E, 512], BF16, kind="ExternalInput")
    wv = nc.dram_tensor("wv", [E, 256], BF16, kind="ExternalInput")
    wo = nc.dram_tensor("wo", [256, E], BF16, kind="ExternalInput")
    bkq = nc.dram_tensor("bkq", [128, 4], F32, kind="ExternalInput")
    out = nc.dram_tensor("out", [S, E], BF16, kind="ExternalOutput")

    with tile.TileContext(nc) as tc:
        import contextlib
        with contextlib.ExitStack() as ctx:
            const = ctx.enter_context(tc.tile_pool(name="const", bufs=1))
            wpool = ctx.enter_context(tc.tile_pool(name="wpool", bufs=1))
            xpool = ctx.enter_context(tc.tile_pool(name="xpool", bufs=1))
            kqpool = ctx.enter_context(tc.tile_pool(name="kqpool", bufs=1))
            vpool = ctx.enter_context(tc.tile_pool(name="vpool", bufs=1))
            sapool = ctx.enter_context(tc.tile_pool(name="sapool", bufs=1))
            pt_pool = ctx.enter_context(tc.tile_pool(name="pt", bufs=34))
            sasb_pool = ctx.enter_context(tc.tile_pool(name="sasb", bufs=2))
            small = ctx.enter_context(tc.tile_pool(name="small", bufs=8))
            ostage = ctx.enter_context(tc.tile_pool(name="ostage", bufs=3))

            # load weights + x first; DMAs split across the SP and ACT hwdge
            # queues so both stream in parallel. Few, large DMAs (issue costs
            # ~0.7-1.2us of sequencer time each); e-tiles 0-3 of wkq/xT land
            # first so the e-major prologue can start after ~2us.
            wkq_sb = wpool.tile([128, ET, 512], BF16)
            xT_sb = xpool.tile([128, ET, S], BF16)
            wv_sb = wpool.tile([128, ET, 256], BF16)
            wkq_r = wkq.rearrange("(n p) f -> p n f", p=128)
            xT_r = xT.rearrange("(n p) s -> p n s", p=128)
            nc.sync.dma_start(wkq_sb[:, 0:4, :], wkq_r[:, 0:4, :])
            nc.scalar.dma_start(wv_sb[:], wv.rearrange("(n p) f -> p n f", p=128))
            nc.scalar.dma_start(xT_sb[:, 0:4, 0:512], xT_r[:, 0:4, 0:512])
            nc.sync.dma_start(xT_sb[:, 4:8, 0:512], xT_r[:, 4:8, 0:512])
            nc.scalar.dma_start(wkq_sb[:, 4:8, :], wkq_r[:, 4:8, :])
            bkq_sb = const.tile([128, 4], F32)
            nc.sync.dma_start(bkq_sb[:], bkq[:])
            for c in range(1, SCH):
                q = nc.sync if c % 2 == 1 else nc.scalar
                q.dma_start(xT_sb[:, :, 512 * c:512 * (c + 1)],
                            xT_r[:, :, 512 * c:512 * (c + 1)])
            wo_sb = wpool.tile([128, 2, E], BF16)
            nc.scalar.dma_start(wo_sb[:], wo.rearrange("(n p) f -> p n f", p=128))

            triu2 = const.tile([128, 2, 128], BF16)
            make_upper_triangular(nc, triu2[:, 0, :], val=1.0, diag=True)
            make_upper_triangular(nc, triu2[:, 1, :], val=1.0, diag=True)
            ident = const.tile([128, 128], BF16)
            make_identity(nc, ident[:])

            kqT_sb = kqpool.tile([128, 4, S], BF16)      # blk: p0k,p0q,p1k,p1q
            vaug_sb = vpool.tile([128, SKT, HPC, 65], BF16)
            nc.vector.memset(vaug_sb[:, :, :, 64:65], 1.0)
            saT_sb = sapool.tile([128, 2, S], BF16)      # dim1: pair

            for _rep in range(reps):
              pools = {}
              def get_pool(name, bufs):
                  if name not in pools:
                      pools[name] = tc.alloc_tile_pool(name=name, bufs=bufs,
                                                       space="PSUM")
                  return pools[name]
              if True:
                # prologue pools (released before the attention loop)
                pre_kq = get_pool("pre_kq", 4)    # [128,512] f32 -> 1 bank x4
                pre_v = get_pool("pre_v", 4)      # [128,256] f32 -> 1 bank x4

                # ---- projection / output units (PE filler) ----
                ot_cur = [None]

                def kq_unit(blk, c, pool, eng=None):
                    ps = pool.tile([128, 512], F32, tag="pj", name="kqps")
                    f0 = 512 * c
                    for e in range(ET):
                        nc.tensor.matmul(
                            ps[:], wkq_sb[:, e, 128 * blk:128 * (blk + 1)],
                            xT_sb[:, e, f0:f0 + 512],
                            start=(e == 0), stop=(e == ET - 1))
                    if eng is nc.scalar:
                        nc.scalar.activation(
                            kqT_sb[:, blk, f0:f0 + 512], ps[:],
                            mybir.ActivationFunctionType.Identity,
                            bias=bkq_sb[:, blk:blk + 1])
                    else:
                        nc.vector.tensor_scalar_add(
                            kqT_sb[:, blk, f0:f0 + 512], ps[:],
                            bkq_sb[:, blk:blk + 1])

                def v_unit(t, pool, eng=None):
                    ps = pool.tile([128, 256], F32, tag="pj", name="vps")
                    for e in range(ET):
                        nc.tensor.matmul(
                            ps[:], xT_sb[:, e, 128 * t:128 * (t + 1)],
                            wv_sb[:, e, :],
                            start=(e == 0), stop=(e == ET - 1))
                    if eng is nc.scalar:
                        nc.scalar.copy(
                            vaug_sb[:, t, :, 0:64],
                            ps[:].rearrange("p (h d) -> p h d", h=HPC))
                    else:
                        nc.vector.tensor_copy(
                            vaug_sb[:, t, :, 0:64],
                            ps[:].rearrange("p (h d) -> p h d", h=HPC))

                def out_unit(t, h, pool, tail=False):
                    # half-tile of the output projection: [128, 512]
                    ps = pool.tile([128, 512], F32, tag="pj", name="ops")
                    f0 = 512 * h
                    nc.tensor.matmul(ps[:],
                                     saT_sb[:, 0, 128 * t:128 * (t + 1)],
                                     wo_sb[:, 0, f0:f0 + 512],
                                     start=True, stop=False)
                    nc.tensor.matmul(ps[:],
                                     saT_sb[:, 1, 128 * t:128 * (t + 1)],
                                     wo_sb[:, 1, f0:f0 + 512],
                                     start=False, stop=True)
                    if h == 0:
                        ot = ostage.tile([128, 2, 512], BF16, tag="ot",
                                         name="ot")
                        ot_cur[0] = ot
                    ot = ot_cur[0]
                    if tail and h == 1:
                        nc.scalar.copy(ot[:, h, :], ps[:])
                    else:
                        nc.vector.tensor_copy(ot[:, h, :], ps[:])
                    if h == 1:
                        q = nc.scalar if tail else nc.sync
                        q.dma_start(
                            out[128 * t:128 * (t + 1), :],
                            ot.rearrange("p n f -> p (n f)"))

                # filler queue (PE work units) + deferred-eviction queue
                filler = []
                evq = []

                def drain(k):
                    for _ in range(k):
                        if filler:
                            filler.pop(0)()

                def pop_ev(k=1):
                    for _ in range(k):
                        if evq:
                            evq.pop(0)()

                # ---- attention segment ----
                # The j-loop only computes scores+exp (ACT-bound); the AV
                # contraction runs later as 4 "waves" (one per 128-row sq
                # block b), emitted as PE filler inside the NEXT segment's
                # j-loop. Each wave owns two psum BANKS (one accumulation
                # group per head) since psum groups are bank-granular.
                def attn_segment(c, p):
                    sq0 = 512 * c
                    kblk, qblk = 2 * p, 2 * p + 1
                    nj = 4 * c + 4
                    pts = []
                    fil0 = len(filler)
                    for j in range(nj):
                        r = j - 4 * c
                        diag = r >= 0
                        off = 128 * r if diag else 0
                        w = 512 - off
                        st = stsa_ps.tile([128, 2, 512], F32, tag="st",
                                          name="st")
                        nc.tensor.matmul(
                            st[:, 0, 0:w],
                            kqT_sb[0:64, kblk, 128 * j:128 * (j + 1)],
                            kqT_sb[0:64, qblk, sq0 + off:sq0 + 512],
                            start=True, stop=True, tile_position=(0, 0))
                        nc.tensor.matmul(
                            st[:, 1, 0:w],
                            kqT_sb[64:128, kblk, 128 * j:128 * (j + 1)],
                            kqT_sb[64:128, qblk, sq0 + off:sq0 + 512],
                            start=True, stop=True, tile_position=(64, 0))
                        pt = pt_pool.tile([128, 2, 512], BF16, tag="pt",
                                          name="pt")
                        nc.scalar.activation(pt[:, :, off:512],
                                             st[:, :, 0:w],
                                             Exp, scale=0.125)
                        if diag:
                            nc.gpsimd.tensor_mul(
                                pt[:, :, off:off + 128],
                                pt[:, :, off:off + 128], triu2[:])
                        pts.append(pt)
                        pop_ev()
                        # pace filler evenly across the segment so the
                        # ACT-backlogged final j's still have PE work
                        want = -(-fil0 * (j + 1) // nj)  # ceil
                        emitted = fil0 - len(filler)
                        drain(max(0, want - emitted) if fil0 else 0)

                    def wave(b):
                        # sa: [128, s, bank] with the per-head accumulation
                        # group in cols 0:65 of its own bank (col 64 = the
                        # softmax row-sums via vaug's ones column)
                        sa = wave_pool[0].tile([128, 2, 512], F32, tag="st",
                                               name="sa")
                        njb = 4 * c + b + 1
                        for s in (0, 1):
                            for j in range(njb):
                                nc.tensor.matmul(
                                    sa[:, s, 0:65],
                                    pts[j][:, s, 128 * b:128 * (b + 1)],
                                    vaug_sb[:, j, 2 * p + s, :],
                                    start=(j == 0), stop=(j == njb - 1))
                        rc = small.tile([128, 2, 1], F32, tag="rc",
                                        name="rc")
                        nc.vector.reciprocal(rc[:], sa[:, :, 64:65])
                        sb = sasb_pool.tile([128, 2, 64], BF16,
                                            tag="sasb", name="sasb")
                        for s in (0, 1):
                            nc.vector.tensor_scalar_mul(
                                sb[:, s, :], sa[:, s, 0:64], rc[:, s, :])
                        tr = tr_ps.tile([128, 128], BF16, tag="tr",
                                        name="tr")
                        nc.tensor.transpose(tr[:], sb[:, :, :], ident[:])
                        nc.vector.tensor_copy(
                            saT_sb[:, p, sq0 + 128 * b:sq0 + 128 * (b + 1)],
                            tr[:])
                    for b in range(4):
                        evq.append(lambda bb=b: wave(bb))

                # ---- schedule ----
                # prologue: chunk-0 projections, e-major so each arriving
                # e-tile feeds 8 matmuls; kq blk0/1 finish + evict first
                # (blk1 via the idle ACT) so the first scores start ASAP
                kps = [pre_kq.tile([128, 512], F32, tag="pj",
                                   name=f"kq{blk}") for blk in range(4)]
                vps = [pre_v.tile([128, 256], F32, tag="pj",
                                  name=f"v{t}") for t in range(4)]
                for e in range(ET):
                    order = range(4) if e < ET - 1 else (0, 1)
                    for blk in order:
                        nc.tensor.matmul(
                            kps[blk][:],
                            wkq_sb[:, e, 128 * blk:128 * (blk + 1)],
                            xT_sb[:, e, 0:512],
                            start=(e == 0), stop=(e == ET - 1))
                    if e < ET - 1:
                        for t in range(4):
                            nc.tensor.matmul(
                                vps[t][:],
                                xT_sb[:, e, 128 * t:128 * (t + 1)],
                                wv_sb[:, e, :],
                                start=(e == 0), stop=False)
                nc.vector.tensor_scalar_add(
                    kqT_sb[:, 0, 0:512], kps[0][:], bkq_sb[:, 0:1])
                nc.scalar.activation(
                    kqT_sb[:, 1, 0:512], kps[1][:],
                    mybir.ActivationFunctionType.Identity,
                    bias=bkq_sb[:, 1:2])
                for blk in (2, 3):
                    nc.tensor.matmul(
                        kps[blk][:],
                        wkq_sb[:, ET - 1, 128 * blk:128 * (blk + 1)],
                        xT_sb[:, ET - 1, 0:512],
                        start=False, stop=True)
                for t in range(4):
                    nc.tensor.matmul(
                        vps[t][:], xT_sb[:, ET - 1, 128 * t:128 * (t + 1)],
                        wv_sb[:, ET - 1, :], start=False, stop=True)
                nc.vector.tensor_scalar_add(
                    kqT_sb[:, 2, 0:512], kps[2][:], bkq_sb[:, 2:3])
                nc.scalar.activation(
                    kqT_sb[:, 3, 0:512], kps[3][:],
                    mybir.ActivationFunctionType.Identity,
                    bias=bkq_sb[:, 3:4])
                for t in range(4):
                    eng_v = nc.vector if t % 2 == 0 else nc.scalar
                    if eng_v is nc.scalar:
                        nc.scalar.copy(
                            vaug_sb[:, t, :, 0:64],
                            vps[t][:].rearrange("p (h d) -> p h d", h=HPC))
                    else:
                        nc.vector.tensor_copy(
                            vaug_sb[:, t, :, 0:64],
                            vps[t][:].rearrange("p (h d) -> p h d", h=HPC))
                pools.pop("pre_v").release()
                pools.pop("pre_kq").release()

                pj_ps = get_pool("pj_ps", 1)     # 1 x 1 bank
                tr_ps = get_pool("tr_ps", 1)     # 1 x 1 bank
                stsa_ps = get_pool("stsa_ps", 3)  # 3 x 2 banks (st + AV waves)
                wave_pool = [stsa_ps]

                for c in range(SCH):
                    if c + 1 < SCH:
                        for blk in range(4):
                            filler.append(
                                lambda b=blk, cc=c + 1:
                                kq_unit(b, cc, pj_ps))
                        for t in range(4 * c + 4, 4 * c + 8):
                            filler.append(
                                lambda tt=t: v_unit(tt, pj_ps))
                        if c == 2:
                            for t in range(0, 4):
                                for h in range(2):
                                    filler.append(
                                        lambda tt=t, hh=h:
                                        out_unit(tt, hh, pj_ps))
                    else:
                        for t in range(4, 8):
                            for h in range(2):
                                filler.append(
                                    lambda tt=t, hh=h:
                                    out_unit(tt, hh, pj_ps))
                    attn_segment(c, 0)
                    if c == 3:
                        for t in range(8, 12):
                            for h in range(2):
                                filler.append(
                                    lambda tt=t, hh=h:
                                    out_unit(tt, hh, pj_ps))
                    attn_segment(c, 1)
                    drain(len(filler))  # anything not yet emitted
                # tail: release the attention pools, redo the last 4 AV waves
                # and output tiles with fat double-buffered pools, waves
                # interleaved with out tiles (tile 12+b only needs wave b)
                pools.pop("stsa_ps").release()
                tail_sa = get_pool("tail_sa", 2)   # [128,2,512] f32 -> 2x2
                tail_pj = get_pool("tail_pj", 2)   # 2 x 1 bank
                wave_pool[0] = tail_sa
                pop_ev()
                for b in range(4):
                    pop_ev()
                    for h in range(2):
                        out_unit(12 + b, h, tail_pj, tail=True)

              for pl in reversed(list(pools)):
                  pools.pop(pl).release()

    nc.compile()
    return nc


_CACHE = {}


def _build_runner():
    """Build the SPMD PJRT executable once; returns a dict with a jitted fn."""
    import jax
    from jax.sharding import Mesh, PartitionSpec
    from jax.experimental.shard_map import shard_map
    from concourse import bass2jax as b2j
    from concourse import mybir as _mybir

    if "runner" in _CACHE:
        return _CACHE["runner"]

    nc = _CACHE.get("nc")
    if nc is None:
        nc = _CACHE["nc"] = build_nc()

    b2j.install_neuronx_cc_hook()
    partition_name = (nc.partition_id_tensor.name
                      if nc.partition_id_tensor else None)

    in_names, out_names, out_avals = [], [], []
    for alloc in nc.m.functions[0].allocations:
        if not isinstance(alloc, _mybir.MemoryLocationSet):
            continue
        name = alloc.memorylocations[0].name
        if alloc.kind == "ExternalInput":
            if name != partition_name:
                in_names.append(name)
        elif alloc.kind == "ExternalOutput":
            out_names.append(name)
            out_avals.append(jax.core.ShapedArray(
                tuple(alloc.tensor_shape), _mybir.dt.np(alloc.dtype)))
    n_params = len(in_names)
    zero_out_shapes = [(a.shape, a.dtype) for a in out_avals]
    all_in_names = list(in_names) + list(out_names)
    if partition_name is not None:
        all_in_names.append(partition_name)

    def _body(*args):
        operands = list(args)
        if partition_name is not None:
            operands.append(b2j.partition_id_tensor())
        outs = b2j._bass_exec_p.bind(
            *operands,
            out_avals=tuple(out_avals),
            in_names=tuple(all_in_names),
            out_names=tuple(out_names),
            lowering_input_output_aliases=(),
            sim_require_finite=True,
            sim_require_nnan=True,
            nc=nc,
        )
        return tuple(outs)

    devices = jax.devices()[:N_CORES]
    mesh = Mesh(np.asarray(devices), ("core",))
    n_outs = len(out_names)
    in_specs = (PartitionSpec("core"),) * (n_params + n_outs)
    out_specs = (PartitionSpec("core"),) * n_outs
    fn = jax.jit(shard_map(_body, mesh=mesh, in_specs=in_specs,
                           out_specs=out_specs, check_rep=False),
                 keep_unused=True)
    runner = {
        "fn": fn,
        "in_names": in_names,
        "out_names": out_names,
        "out_avals": out_avals,
        "zero_out_shapes": zero_out_shapes,
        "mesh": mesh,
    }
    _CACHE["runner"] = runner
    return runner


def _run_spmd(in_maps):
    """Execute on 8 cores, returning list of per-core output dicts."""
    r = _build_runner()
    n_cores = N_CORES
    concat_in = [
        np.concatenate([np.asarray(in_maps[c][name]) for c in range(n_cores)],
                       axis=0)
        for name in r["in_names"]
    ]
    if "zeros" not in r:
        r["zeros"] = [np.zeros((n_cores * s[0], *s[1:]), d)
                      for s, d in r["zero_out_shapes"]]
    out_arrs = r["fn"](*concat_in, *r["zeros"])
    return [
        {name: np.asarray(out_arrs[i]).reshape(n_cores, *r["out_avals"][i].shape)[c]
         for i, name in enumerate(r["out_names"])}
        for c in range(n_cores)
    ]


def _prep_core_inputs(x, Wkqv, bkqv, Wo):
    """Host-side shard/pack. Returns (in_maps, host_bias) for 8 cores."""
    xT = [np.ascontiguousarray(x[b].T).astype(NP_BF16) for b in range(B)]
    per_g = []
    for g in range(4):
        h0 = 4 * g
        wkq = np.empty((E, 512), np.float32)
        for p in range(2):
            a, b_ = h0 + 2 * p, h0 + 2 * p + 1
            wkq[:, 256 * p:256 * p + 64] = Wkqv[a][:, 0:64]
            wkq[:, 256 * p + 64:256 * p + 128] = Wkqv[b_][:, 0:64]
            wkq[:, 256 * p + 128:256 * p + 192] = Wkqv[a][:, 64:128]
            wkq[:, 256 * p + 192:256 * p + 256] = Wkqv[b_][:, 64:128]
        wv = np.concatenate([Wkqv[h0 + h][:, 128:192] for h in range(HPC)],
                            axis=1)
        wog = Wo[256 * g:256 * (g + 1), :]
        bkq_arr = np.empty((128, 4), np.float32)
        for p in range(2):
            a, b_ = h0 + 2 * p, h0 + 2 * p + 1
            bkq_arr[0:64, 2 * p] = bkqv[a][0:64]
            bkq_arr[64:128, 2 * p] = bkqv[b_][0:64]
            bkq_arr[0:64, 2 * p + 1] = bkqv[a][64:128]
            bkq_arr[64:128, 2 * p + 1] = bkqv[b_][64:128]
        per_g.append({
            "wkq": wkq.astype(NP_BF16),
            "wv": wv.astype(NP_BF16),
            "wo": wog.astype(NP_BF16),
            "bkq": bkq_arr,
        })
    in_maps = []
    for c in range(N_CORES):
        b, g = c // 4, c % 4
        m = dict(per_g[g])
        m["xT"] = xT[b]
        in_maps.append(m)
    bv = np.concatenate([bkqv[h][128:192] for h in range(H)])
    return in_maps, bv


def kernel(x, Wkqv, bkqv, Wo, bo):
    x = np.asarray(x, np.float32)
    Wkqv = np.asarray(Wkqv, np.float32)
    bkqv = np.asarray(bkqv, np.float32)
    Wo = np.asarray(Wo, np.float32)
    bo = np.asarray(bo, np.float32)

    in_maps, bv = _prep_core_inputs(x, Wkqv, bkqv, Wo)
    results = _run_spmd(in_maps)
    partials = np.stack([results[c]["out"] for c in range(N_CORES)]).astype(np.float32)
    partials = partials.reshape(B, 4, S, E).sum(axis=1)
    base = bv @ Wo + bo
    return (partials + base[None, None, :]).astype(np.float32)


# revision 56
# speedup vs baseline: 1.0123x; 1.0043x over previous
"""Causal self-attention (B=2, S=2048, E=1024, H=16, DH=64) on 8 trn2 cores.

Sharding: core c -> (batch b = c//4, head-group g = c%4, heads 4g..4g+3).
Data parallel over batch, tensor parallel over heads, row-sharded Wo;
partial outputs summed on host.

Per-core device kernel (bf16 matmuls, fp32 accumulation), fully software-
pipelined so the PE never starves while the ACT engine grinds through exp:

  - scores^T[sk,sq] = k q^T per (512-wide sq chunk c, head-pair p, sk tile j),
    2 heads row-packed in the PE via tile_position (K=64); the j-loop emits
    ONLY scores+exp (ACT-bound) and keeps all pt = exp(scores/8) tiles of a
    segment resident in SBUF (causal-trimmed, triu diag masks on Pool).
  - The AV contraction runs as 4 deferred "waves" per segment (one per
    128-row sq block b), emitted as PE filler inside the NEXT segment's
    j-loop: sa[128sq, 65] += pt-block^T @ v_aug with pt stationary (M=128
    fully used, N=65 -> MAC-optimal, half the cost of the v-stationary
    form; col 64 accumulates softmax row-sums via v_aug's ones column).
    Each wave owns two whole psum BANKS (psum accumulation groups are
    2KB-bank-granular). Eviction: per-partition reciprocal of the row-sums,
    normalize-on-evict (DVE tensor_scalar), one PE-transpose per block
    (2 heads packed) back to feature-major saT.
  - Projections (kqT = (x Wkq)^T + b, v = x Wv) and output tiles
    (out[t] = saT^T @ Wo row-shard, bf16) are emitted as paced PE filler
    units between attention j-iterations; an e-major prologue overlaps the
    first chunk's projections with the input DMAs (split across the SP and
    ACT hwdge queues); the tail interleaves the last AV waves with the
    final output tiles on fat double-buffered pools.
"""
import numpy as np
import ml_dtypes

import concourse.bass as bass
import concourse.bacc as bacc
import concourse.tile as tile
from concourse import mybir
from concourse.masks import make_upper_triangular, make_identity

BF16 = mybir.dt.bfloat16
F32 = mybir.dt.float32
NP_BF16 = ml_dtypes.bfloat16

B, S, E, H, DH = 2, 2048, 1024, 16, 64
N_CORES = 8
HPC = 4          # heads per core
SCH = 4          # number of 512-wide sq chunks
SKT = 16         # number of 128-wide sk tiles
ET = 8           # number of 128-wide e tiles

Exp = mybir.ActivationFunctionType.Exp
Copy = mybir.ActivationFunctionType.Copy


def build_nc(reps=1):
    nc = bacc.Bacc(None, target_bir_lowering=False)

    xT = nc.dram_tensor("xT", [E, S], BF16, kind="ExternalInput")
    wkq = nc.dram_tensor("wkq", [E, 512], BF16, kind="ExternalInput")
    wv = nc.dram_tensor("wv", [E, 256], BF16, kind="ExternalInput")
    wo = nc.dram_tensor("wo", [256, E], BF16, kind="ExternalInput")
    bkq = nc.dram_tensor("bkq", [128, 4], F32, kind="ExternalInput")
    out = nc.dram_tensor("out", [S, E], BF16, kind="ExternalOutput")

    with tile.TileContext(nc) as tc:
        import contextlib
        with contextlib.ExitStack() as ctx:
            const = ctx.enter_context(tc.tile_pool(name="const", bufs=1))
            wpool = ctx.enter_context(tc.tile_pool(name="wpool", bufs=1))
            xpool = ctx.enter_context(tc.tile_pool(name="xpool", bufs=1))
            kqpool = ctx.enter_context(tc.tile_pool(name="kqpool", bufs=1))
            vpool = ctx.enter_context(tc.tile_pool(name="vpool", bufs=1))
            sapool = ctx.enter_context(tc.tile_pool(name="sapool", bufs=1))
            pt_pool = ctx.enter_context(tc.tile_pool(name="pt", bufs=34))
            sasb_pool = ctx.enter_context(tc.tile_pool(name="sasb", bufs=4))
            small = ctx.enter_context(tc.tile_pool(name="small", bufs=8))
            ostage = ctx.enter_context(tc.tile_pool(name="ostage", bufs=4))

            # load weights + x first; DMAs split across the SP and ACT hwdge
            # queues so both stream in parallel. Few, large DMAs (issue costs
            # ~0.7-1.2us of sequencer time each); e-tiles 0-3 of wkq/xT land
            # first so the e-major prologue can start after ~2us.
            wkq_sb = wpool.tile([128, ET, 512], BF16)
            xT_sb = xpool.tile([128, ET, S], BF16)
            wv_sb = wpool.tile([128, ET, 256], BF16)
            wkq_r = wkq.rearrange("(n p) f -> p n f", p=128)
            xT_r = xT.rearrange("(n p) s -> p n s", p=128)
            nc.sync.dma_start(wkq_sb[:, 0:4, :], wkq_r[:, 0:4, :])
            nc.scalar.dma_start(wv_sb[:], wv.rearrange("(n p) f -> p n f", p=128))
            nc.scalar.dma_start(xT_sb[:, 0:4, 0:512], xT_r[:, 0:4, 0:512])
            nc.sync.dma_start(xT_sb[:, 4:8, 0:512], xT_r[:, 4:8, 0:512])
            nc.scalar.dma_start(wkq_sb[:, 4:8, :], wkq_r[:, 4:8, :])
            bkq_sb = const.tile([128, 4], F32)
            nc.sync.dma_start(bkq_sb[:], bkq[:])
            for c in range(1, SCH):
                q = nc.sync if c % 2 == 1 else nc.scalar
                q.dma_start(xT_sb[:, :, 512 * c:512 * (c + 1)],
                            xT_r[:, :, 512 * c:512 * (c + 1)])
            wo_sb = wpool.tile([128, 2, E], BF16)
            nc.scalar.dma_start(wo_sb[:], wo.rearrange("(n p) f -> p n f", p=128))

            triu2 = const.tile([128, 2, 128], BF16)
            make_upper_triangular(nc, triu2[:, 0, :], val=1.0, diag=True)
            make_upper_triangular(nc, triu2[:, 1, :], val=1.0, diag=True)
            ident = const.tile([128, 128], BF16)
            make_identity(nc, ident[:])

            kqT_sb = kqpool.tile([128, 4, S], BF16)      # blk: p0k,p0q,p1k,p1q
            vaug_sb = vpool.tile([128, SKT, HPC, 65], BF16)
            nc.vector.memset(vaug_sb[:, :, :, 64:65], 1.0)
            saT_sb = sapool.tile([128, 2, S], BF16)      # dim1: pair

            for _rep in range(reps):
              pools = {}
              def get_pool(name, bufs):
                  if name not in pools:
                      pools[name] = tc.alloc_tile_pool(name=name, bufs=bufs,
                                                       space="PSUM")
                  return pools[name]
              if True:
                # prologue pools (released before the attention loop)
                pre_kq = get_pool("pre_kq", 4)    # [128,512] f32 -> 1 bank x4
                pre_v = get_pool("pre_v", 4)      # [128,256] f32 -> 1 bank x4

                # ---- projection / output units (PE filler) ----
                ot_cur = [None]

                def kq_unit(blk, c, pool, eng=None):
                    ps = pool.tile([128, 512], F32, tag="pj", name="kqps")
                    f0 = 512 * c
                    for e in range(ET):
                        nc.tensor.matmul(
                            ps[:], wkq_sb[:, e, 128 * blk:128 * (blk + 1)],
                            xT_sb[:, e, f0:f0 + 512],
                            start=(e == 0), stop=(e == ET - 1))
                    if eng is nc.scalar:
                        nc.scalar.activation(
                            kqT_sb[:, blk, f0:f0 + 512], ps[:],
                            mybir.ActivationFunctionType.Identity,
                            bias=bkq_sb[:, blk:blk + 1])
                    else:
                        nc.vector.tensor_scalar_add(
                            kqT_sb[:, blk, f0:f0 + 512], ps[:],
                            bkq_sb[:, blk:blk + 1])

                def v_unit(t, pool, eng=None):
                    ps = pool.tile([128, 256], F32, tag="pj", name="vps")
                    for e in range(ET):
                        nc.tensor.matmul(
                            ps[:], xT_sb[:, e, 128 * t:128 * (t + 1)],
                            wv_sb[:, e, :],
                            start=(e == 0), stop=(e == ET - 1))
                    if eng is nc.scalar:
                        nc.scalar.copy(
                            vaug_sb[:, t, :, 0:64],
                            ps[:].rearrange("p (h d) -> p h d", h=HPC))
                    else:
                        nc.vector.tensor_copy(
                            vaug_sb[:, t, :, 0:64],
                            ps[:].rearrange("p (h d) -> p h d", h=HPC))

                def out_unit(t, h, pool, tail=False):
                    # half-tile of the output projection: [128, 512]
                    ps = pool.tile([128, 512], F32, tag="pj", name="ops")
                    f0 = 512 * h
                    nc.tensor.matmul(ps[:],
                                     saT_sb[:, 0, 128 * t:128 * (t + 1)],
                                     wo_sb[:, 0, f0:f0 + 512],
                                     start=True, stop=False)
                    nc.tensor.matmul(ps[:],
                                     saT_sb[:, 1, 128 * t:128 * (t + 1)],
                                     wo_sb[:, 1, f0:f0 + 512],
                                     start=False, stop=True)
                    if h == 0:
                        ot = ostage.tile([128, 2, 512], BF16, tag="ot",
                                         name="ot")
                        ot_cur[0] = ot
                    ot = ot_cur[0]
                    if tail and h == 1:
                        nc.scalar.copy(ot[:, h, :], ps[:])
                    else:
                        nc.vector.tensor_copy(ot[:, h, :], ps[:])
                    if h == 1:
                        q = nc.scalar if tail else nc.sync
                        q.dma_start(
                            out[128 * t:128 * (t + 1), :],
                            ot.rearrange("p n f -> p (n f)"))

                # filler queue (PE work units) + deferred-eviction queue
                filler = []
                evq = []

                def drain(k):
                    for _ in range(k):
                        if filler:
                            filler.pop(0)()

                def pop_ev(k=1):
                    for _ in range(k):
                        if evq:
                            evq.pop(0)()

                # ---- attention segment ----
                # The j-loop only computes scores+exp (ACT-bound); the AV
                # contraction runs later as 4 "waves" (one per 128-row sq
                # block b), emitted as PE filler inside the NEXT segment's
                # j-loop. Each wave owns two psum BANKS (one accumulation
                # group per head) since psum groups are bank-granular.
                def attn_segment(c, p):
                    sq0 = 512 * c
                    kblk, qblk = 2 * p, 2 * p + 1
                    nj = 4 * c + 4
                    pts = []
                    fil0 = len(filler)
                    for j in range(nj):
                        r = j - 4 * c
                        diag = r >= 0
                        off = 128 * r if diag else 0
                        w = 512 - off
                        st = stsa_ps.tile([128, 2, 512], F32, tag="st",
                                          name="st")
                        nc.tensor.matmul(
                            st[:, 0, 0:w],
                            kqT_sb[0:64, kblk, 128 * j:128 * (j + 1)],
                            kqT_sb[0:64, qblk, sq0 + off:sq0 + 512],
                            start=True, stop=True, tile_position=(0, 0))
                        nc.tensor.matmul(
                            st[:, 1, 0:w],
                            kqT_sb[64:128, kblk, 128 * j:128 * (j + 1)],
                            kqT_sb[64:128, qblk, sq0 + off:sq0 + 512],
                            start=True, stop=True, tile_position=(64, 0))
                        pt = pt_pool.tile([128, 2, 512], BF16, tag="pt",
                                          name="pt")
                        nc.scalar.activation(pt[:, :, off:512],
                                             st[:, :, 0:w],
                                             Exp, scale=0.125)
                        if diag:
                            nc.gpsimd.tensor_mul(
                                pt[:, :, off:off + 128],
                                pt[:, :, off:off + 128], triu2[:])
                        pts.append(pt)
                        pop_ev()
                        # pace filler evenly across the segment so the
                        # ACT-backlogged final j's still have PE work
                        want = -(-fil0 * (j + 1) // nj)  # ceil
                        emitted = fil0 - len(filler)
                        drain(max(0, want - emitted) if fil0 else 0)

                    def wave(b):
                        # sa: [128, s, bank] with the per-head accumulation
                        # group in cols 0:65 of its own bank (col 64 = the
                        # softmax row-sums via vaug's ones column)
                        sa = wave_pool[0].tile([128, 2, 512], F32, tag="st",
                                               name="sa")
                        njb = 4 * c + b + 1
                        for s in (0, 1):
                            for j in range(njb):
                                nc.tensor.matmul(
                                    sa[:, s, 0:65],
                                    pts[j][:, s, 128 * b:128 * (b + 1)],
                                    vaug_sb[:, j, 2 * p + s, :],
                                    start=(j == 0), stop=(j == njb - 1))
                        rc = small.tile([128, 2, 1], F32, tag="rc",
                                        name="rc")
                        nc.vector.reciprocal(rc[:], sa[:, :, 64:65])
                        sb = sasb_pool.tile([128, 2, 64], BF16,
                                            tag="sasb", name="sasb")
                        for s in (0, 1):
                            nc.vector.tensor_scalar_mul(
                                sb[:, s, :], sa[:, s, 0:64], rc[:, s, :])
                        tr = tr_ps.tile([128, 128], BF16, tag="tr",
                                        name="tr")
                        nc.tensor.transpose(tr[:], sb[:, :, :], ident[:])
                        nc.vector.tensor_copy(
                            saT_sb[:, p, sq0 + 128 * b:sq0 + 128 * (b + 1)],
                            tr[:])
                    for b in range(4):
                        evq.append(lambda bb=b: wave(bb))

                # ---- schedule ----
                # prologue: chunk-0 projections, e-major so each arriving
                # e-tile feeds 8 matmuls; kq blk0/1 finish + evict first
                # (blk1 via the idle ACT) so the first scores start ASAP
                kps = [pre_kq.tile([128, 512], F32, tag="pj",
                                   name=f"kq{blk}") for blk in range(4)]
                vps = [pre_v.tile([128, 256], F32, tag="pj",
                                  name=f"v{t}") for t in range(4)]
                for e in range(ET):
                    order = range(4) if e < ET - 1 else (0, 1)
                    for blk in order:
                        nc.tensor.matmul(
                            kps[blk][:],
                            wkq_sb[:, e, 128 * blk:128 * (blk + 1)],
                            xT_sb[:, e, 0:512],
                            start=(e == 0), stop=(e == ET - 1))
                    if e < ET - 1:
                        for t in range(4):
                            nc.tensor.matmul(
                                vps[t][:],
                                xT_sb[:, e, 128 * t:128 * (t + 1)],
                                wv_sb[:, e, :],
                                start=(e == 0), stop=False)
                nc.vector.tensor_scalar_add(
                    kqT_sb[:, 0, 0:512], kps[0][:], bkq_sb[:, 0:1])
                nc.scalar.activation(
                    kqT_sb[:, 1, 0:512], kps[1][:],
                    mybir.ActivationFunctionType.Identity,
                    bias=bkq_sb[:, 1:2])
                for blk in (2, 3):
                    nc.tensor.matmul(
                        kps[blk][:],
                        wkq_sb[:, ET - 1, 128 * blk:128 * (blk + 1)],
                        xT_sb[:, ET - 1, 0:512],
                        start=False, stop=True)
                for t in range(4):
                    nc.tensor.matmul(
                        vps[t][:], xT_sb[:, ET - 1, 128 * t:128 * (t + 1)],
                        wv_sb[:, ET - 1, :], start=False, stop=True)
                nc.vector.tensor_scalar_add(
                    kqT_sb[:, 2, 0:512], kps[2][:], bkq_sb[:, 2:3])
                nc.scalar.activation(
                    kqT_sb[:, 3, 0:512], kps[3][:],
                    mybir.ActivationFunctionType.Identity,
                    bias=bkq_sb[:, 3:4])
                for t in range(4):
                    eng_v = nc.vector if t % 2 == 0 else nc.scalar
                    if eng_v is nc.scalar:
                        nc.scalar.copy(
                            vaug_sb[:, t, :, 0:64],
                            vps[t][:].rearrange("p (h d) -> p h d", h=HPC))
                    else:
                        nc.vector.tensor_copy(
                            vaug_sb[:, t, :, 0:64],
                            vps[t][:].rearrange("p (h d) -> p h d", h=HPC))
                pools.pop("pre_v").release()
                pools.pop("pre_kq").release()

                pj_ps = get_pool("pj_ps", 1)     # 1 x 1 bank
                tr_ps = get_pool("tr_ps", 1)     # 1 x 1 bank
                stsa_ps = get_pool("stsa_ps", 3)  # 3 x 2 banks (st + AV waves)
                wave_pool = [stsa_ps]

                for c in range(SCH):
                    if c + 1 < SCH:
                        for blk in range(4):
                            filler.append(
                                lambda b=blk, cc=c + 1:
                                kq_unit(b, cc, pj_ps))
                        for t in range(4 * c + 4, 4 * c + 8):
                            filler.append(
                                lambda tt=t: v_unit(tt, pj_ps))
                        if c == 2:
                            for t in range(0, 4):
                                for h in range(2):
                                    filler.append(
                                        lambda tt=t, hh=h:
                                        out_unit(tt, hh, pj_ps))
                    else:
                        for t in range(4, 8):
                            for h in range(2):
                                filler.append(
                                    lambda tt=t, hh=h:
                                    out_unit(tt, hh, pj_ps))
                    attn_segment(c, 0)
                    if c == 3:
                        for t in range(8, 12):
                            for h in range(2):
                                filler.append(
                                    lambda tt=t, hh=h:
                                    out_unit(tt, hh, pj_ps))
                    attn_segment(c, 1)
                    drain(len(filler))  # anything not yet emitted
                # tail: release the attention pools, redo the last 4 AV waves
                # and output tiles with fat double-buffered pools, waves
                # interleaved with out tiles (tile 12+b only needs wave b)
                pools.pop("stsa_ps").release()
                tail_sa = get_pool("tail_sa", 2)   # [128,2,512] f32 -> 2x2
                tail_pj = get_pool("tail_pj", 2)   # 2 x 1 bank
                wave_pool[0] = tail_sa
                pop_ev()
                for b in range(4):
                    pop_ev()
                    for h in range(2):
                        out_unit(12 + b, h, tail_pj, tail=True)

              for pl in reversed(list(pools)):
                  pools.pop(pl).release()

    nc.compile()
    return nc


_CACHE = {}


def _build_runner():
    """Build the SPMD PJRT executable once; returns a dict with a jitted fn."""
    import jax
    from jax.sharding import Mesh, PartitionSpec
    from jax.experimental.shard_map import shard_map
    from concourse import bass2jax as b2j
    from concourse import mybir as _mybir

    if "runner" in _CACHE:
        return _CACHE["runner"]

    nc = _CACHE.get("nc")
    if nc is None:
        nc = _CACHE["nc"] = build_nc()

    b2j.install_neuronx_cc_hook()
    partition_name = (nc.partition_id_tensor.name
                      if nc.partition_id_tensor else None)

    in_names, out_names, out_avals = [], [], []
    for alloc in nc.m.functions[0].allocations:
        if not isinstance(alloc, _mybir.MemoryLocationSet):
            continue
        name = alloc.memorylocations[0].name
        if alloc.kind == "ExternalInput":
            if name != partition_name:
                in_names.append(name)
        elif alloc.kind == "ExternalOutput":
            out_names.append(name)
            out_avals.append(jax.core.ShapedArray(
                tuple(alloc.tensor_shape), _mybir.dt.np(alloc.dtype)))
    n_params = len(in_names)
    zero_out_shapes = [(a.shape, a.dtype) for a in out_avals]
    all_in_names = list(in_names) + list(out_names)
    if partition_name is not None:
        all_in_names.append(partition_name)

    def _body(*args):
        operands = list(args)
        if partition_name is not None:
            operands.append(b2j.partition_id_tensor())
        outs = b2j._bass_exec_p.bind(
            *operands,
            out_avals=tuple(out_avals),
            in_names=tuple(all_in_names),
            out_names=tuple(out_names),
            lowering_input_output_aliases=(),
            sim_require_finite=True,
            sim_require_nnan=True,
            nc=nc,
        )
        return tuple(outs)

    devices = jax.devices()[:N_CORES]
    mesh = Mesh(np.asarray(devices), ("core",))
    n_outs = len(out_names)
    in_specs = (PartitionSpec("core"),) * (n_params + n_outs)
    out_specs = (PartitionSpec("core"),) * n_outs
    fn = jax.jit(shard_map(_body, mesh=mesh, in_specs=in_specs,
                           out_specs=out_specs, check_rep=False),
                 keep_unused=True)
    runner = {
        "fn": fn,
        "in_names": in_names,
        "out_names": out_names,
        "out_avals": out_avals,
        "zero_out_shapes": zero_out_shapes,
        "mesh": mesh,
    }
    _CACHE["runner"] = runner
    return runner


def _run_spmd(in_maps):
    """Execute on 8 cores, returning list of per-core output dicts."""
    r = _build_runner()
    n_cores = N_CORES
    concat_in = [
        np.concatenate([np.asarray(in_maps[c][name]) for c in range(n_cores)],
                       axis=0)
        for name in r["in_names"]
    ]
    if "zeros" not in r:
        r["zeros"] = [np.zeros((n_cores * s[0], *s[1:]), d)
                      for s, d in r["zero_out_shapes"]]
    out_arrs = r["fn"](*concat_in, *r["zeros"])
    return [
        {name: np.asarray(out_arrs[i]).reshape(n_cores, *r["out_avals"][i].shape)[c]
         for i, name in enumerate(r["out_names"])}
        for c in range(n_cores)
    ]


def _prep_core_inputs(x, Wkqv, bkqv, Wo):
    """Host-side shard/pack. Returns (in_maps, host_bias) for 8 cores."""
    xT = [np.ascontiguousarray(x[b].T).astype(NP_BF16) for b in range(B)]
    per_g = []
    for g in range(4):
        h0 = 4 * g
        wkq = np.empty((E, 512), np.float32)
        for p in range(2):
            a, b_ = h0 + 2 * p, h0 + 2 * p + 1
            wkq[:, 256 * p:256 * p + 64] = Wkqv[a][:, 0:64]
            wkq[:, 256 * p + 64:256 * p + 128] = Wkqv[b_][:, 0:64]
            wkq[:, 256 * p + 128:256 * p + 192] = Wkqv[a][:, 64:128]
            wkq[:, 256 * p + 192:256 * p + 256] = Wkqv[b_][:, 64:128]
        wv = np.concatenate([Wkqv[h0 + h][:, 128:192] for h in range(HPC)],
                            axis=1)
        wog = Wo[256 * g:256 * (g + 1), :]
        bkq_arr = np.empty((128, 4), np.float32)
        for p in range(2):
            a, b_ = h0 + 2 * p, h0 + 2 * p + 1
            bkq_arr[0:64, 2 * p] = bkqv[a][0:64]
            bkq_arr[64:128, 2 * p] = bkqv[b_][0:64]
            bkq_arr[0:64, 2 * p + 1] = bkqv[a][64:128]
            bkq_arr[64:128, 2 * p + 1] = bkqv[b_][64:128]
        per_g.append({
            "wkq": wkq.astype(NP_BF16),
            "wv": wv.astype(NP_BF16),
            "wo": wog.astype(NP_BF16),
            "bkq": bkq_arr,
        })
    in_maps = []
    for c in range(N_CORES):
        b, g = c // 4, c % 4
        m = dict(per_g[g])
        m["xT"] = xT[b]
        in_maps.append(m)
    bv = np.concatenate([bkqv[h][128:192] for h in range(H)])
    return in_maps, bv


def kernel(x, Wkqv, bkqv, Wo, bo):
    x = np.asarray(x, np.float32)
    Wkqv = np.asarray(Wkqv, np.float32)
    bkqv = np.asarray(bkqv, np.float32)
    Wo = np.asarray(Wo, np.float32)
    bo = np.asarray(bo, np.float32)

    in_maps, bv = _prep_core_inputs(x, Wkqv, bkqv, Wo)
    results = _run_spmd(in_maps)
    partials = np.stack([results[c]["out"] for c in range(N_CORES)]).astype(np.float32)
    partials = partials.reshape(B, 4, S, E).sum(axis=1)
    base = bv @ Wo + bo
    return (partials + base[None, None, :]).astype(np.float32)
